# revision 1
# baseline (speedup 1.0000x reference)
"""Trainium2 Bass kernel for nn_Decoder_74835510165950 (sparse_attention).

Single-query attention decoder over B=64, N=2000, H=128, 8 heads.
Data-parallel over 8 NeuronCores: 8 batches per core.

Algebraic restructuring (q_len = 1 makes K/V materialization useless):
  scores[b,h,n] = X_b[n,:] @ R_b[:,h]      with R_b = Wk^T @ (blockdiag q~_b)
  attn_out u    = X_b^T @ attn_b           then per-head Wv fold
  pointer[b,n]  = X_b[n,:] @ w_b           with w_b = logit_Wk^T @ fq_b / sqrt(H)
so per core we read X once (8 MB) and run three PE streams over it.

Layout per core (b = 0..7 -> quad q = b // 4, bi = b % 4):
  score/pointer PSUM tiles [128, 2000]: batch bi occupies rows 32*bi..32*bi+8
  (engine APs require 32-aligned partition bases). -1e9 masks are folded into
  the PSUM accumulation via one extra matmul with a host one-hot rhs.
fp32r (TF32-class, 1 cyc/row) for big streams; fp32 for small matmuls.
"""
import sys

if "/opt/trn_rl_repo" not in sys.path:
    sys.path.insert(0, "/opt/trn_rl_repo")

import math
import numpy as np

import concourse.bass as bass
import concourse.tile as tile
from concourse import bacc, mybir
from concourse.bass_utils import run_bass_kernel_spmd

F32 = mybir.dt.float32
F32R = mybir.dt.float32r
BF16 = mybir.dt.bfloat16
I32 = mybir.dt.int32

N_CORES = 8
B_CORE = 8          # batches per core
N = 2000
H = 128
NH = 8              # heads
HD = 16             # head dim
SCHUNKS = [(0, 512), (512, 512), (1024, 512), (1536, 464)]  # bank-aligned
NCHUNK = 500        # xT copy chunk
NJ = 16             # n-chunks per batch
NP = 125            # rows per n-chunk (16 * 125 = 2000)

_CACHE = {}


def r(ap):
    return ap.bitcast(F32R)


def build():
    nc = bacc.Bacc("TRN2", target_bir_lowering=False, debug=False)

    x = nc.dram_tensor("x", [B_CORE, N, H], F32, kind="ExternalInput")
    clsT = nc.dram_tensor("clsT", [H, B_CORE], F32, kind="ExternalInput")
    wqgT = nc.dram_tensor("wqgT", [H, H], F32, kind="ExternalInput")
    wsumT = nc.dram_tensor("wsumT", [H, H], F32, kind="ExternalInput")
    wk = nc.dram_tensor("wk", [H, H], F32, kind="ExternalInput")
    wvT = nc.dram_tensor("wvT", [H, H], F32, kind="ExternalInput")
    wcT = nc.dram_tensor("wcT", [H, H], F32, kind="ExternalInput")
    wlk = nc.dram_tensor("wlk", [H, H], F32, kind="ExternalInput")
    bc = nc.dram_tensor("bc", [H, 1], F32, kind="ExternalInput")
    hm = nc.dram_tensor("hm", [H, NH], F32, kind="ExternalInput")
    identd = nc.dram_tensor("identd", [H, H], F32, kind="ExternalInput")
    oh = nc.dram_tensor("oh", [4, 2, N], F32, kind="ExternalInput")
    mnegA = nc.dram_tensor("mnegA", [4, H], F32, kind="ExternalInput")
    mnegP = nc.dram_tensor("mnegP", [4, H], F32, kind="ExternalInput")
    zer = nc.dram_tensor("zer", [H, 512], F32, kind="ExternalInput")
    seld = nc.dram_tensor("seld", [H, 32], F32, kind="ExternalInput")
    roffs = nc.dram_tensor("roffs", [B_CORE, 1], I32, kind="ExternalInput")

    probs = nc.dram_tensor("probs", [B_CORE, N], F32, kind="ExternalOutput")

    with tile.TileContext(nc) as tc:
        with (
            tc.tile_pool(name="wts", bufs=1) as wts,
            tc.tile_pool(name="xn", bufs=1) as xnp,
            tc.tile_pool(name="xt", bufs=1) as xtp,
            tc.tile_pool(name="big", bufs=4) as bigp,
            tc.tile_pool(name="et", bufs=2) as etp,
            tc.tile_pool(name="pad", bufs=2) as padp,
            tc.tile_pool(name="sm", bufs=1) as smp,
            tc.tile_pool(name="ps_big", bufs=1, space="PSUM") as psb,
            tc.tile_pool(name="ps_tr", bufs=2, space="PSUM") as pst,
            tc.tile_pool(name="ps_sm", bufs=2, space="PSUM") as pss,
        ):
            # ---------- weights & constants ----------
            def wtile(dram, shape, dtype=F32, cast_r=False, tag=None):
                t = wts.tile(shape, dtype, tag=tag or dram.name)
                if cast_r:
                    nc.sync.dma_start(r(t[:]), r(dram[:]))
                else:
                    nc.sync.dma_start(t[:], dram[:])
                return t

            wqgT_s = wtile(wqgT, [H, H])
            wsumT_s = wtile(wsumT, [H, H])
            wk_s = wtile(wk, [H, H])
            wvT_s = wtile(wvT, [H, H])
            wcT_s = wtile(wcT, [H, H])
            wlk_s = wtile(wlk, [H, H])
            bc_s = wtile(bc, [H, 1])
            hm_s = wtile(hm, [H, NH])
            id_f = wtile(identd, [H, H], tag="id_f")
            clsT_s = wtile(clsT, [H, B_CORE])
            oh_s = wtile(oh, [4, 2, N], cast_r=True)
            mnegA_s = wtile(mnegA, [4, H], cast_r=True)
            mnegP_s = wtile(mnegP, [4, H], cast_r=True)
            sel_s = wtile(seld, [H, 32])

            # ---------- X natural, quad-interleaved ----------
            # xn_q[p, j, bi, c] = x[4q + bi, j*125 + p, c]
            xn = []
            for q in range(2):
                t = xnp.tile([NP, NJ, 4 * H], F32, tag=f"xn{q}")
                for bi in range(4):
                    b = 4 * q + bi
                    nc.sync.dma_start(
                        r(t[:, :, H * bi:H * (bi + 1)]),
                        r(x[b].rearrange("(j p) c -> p j c", p=NP)),
                    )
                xn.append(t)

            # ---------- last-patch gather ----------
            roffs_s = smp.tile([B_CORE, 1], I32, tag="roffs")
            nc.sync.dma_start(roffs_s[:], roffs[:])
            le_s = smp.tile([B_CORE, H], F32, tag="le")
            nc.gpsimd.indirect_dma_start(
                out=le_s[:], out_offset=None,
                in_=x[:].rearrange("b n c -> (b n) c"),
                in_offset=bass.IndirectOffsetOnAxis(ap=roffs_s[:, :1], axis=0),
            )

            # ---------- X^T via PE transposes ----------
            xT = xtp.tile([H, B_CORE * N], F32, tag="xT")
            ncopies = 0
            for q in range(2):
                for bi in range(4):
                    b = 4 * q + bi
                    for k in range(4):
                        ps = pst.tile([H, 4, H], F32, tag="trps")
                        for i in range(4):
                            j = 4 * k + i
                            nc.tensor.transpose(
                                ps[:, i, 0:NP],
                                xn[q][0:NP, j, H * bi:H * (bi + 1)].bitcast(F32),
                                id_f[0:NP, 0:NP],
                            )
                        dst = r(xT[:, b * N + NCHUNK * k: b * N + NCHUNK * (k + 1)]
                                .rearrange("p (j n) -> p j n", n=NP))
                        src = ps[:].rearrange("p j c -> p j c")[:, :, 0:NP]
                        nc.vector.tensor_copy(dst, src)
                        ncopies += 1

            # ---------- Q path ----------
            leT_ps = pss.tile([H, B_CORE], F32, tag="smps")
            nc.tensor.transpose(leT_ps[:], le_s[:], id_f[0:B_CORE, 0:B_CORE])
            leT_s = smp.tile([H, B_CORE], F32, tag="leTs")
            nc.vector.tensor_copy(leT_s[:], leT_ps[:])

            q_ps = pss.tile([H, B_CORE], F32, tag="smps")
            nc.tensor.matmul(q_ps[:], wqgT_s[:], clsT_s[:], start=True, stop=False)
            nc.tensor.matmul(q_ps[:], wsumT_s[:], leT_s[:], start=False, stop=True)
            qT_s = smp.tile([H, B_CORE], F32, tag="qTs")
            nc.vector.tensor_copy(qT_s[:], q_ps[:])

            qtil = smp.tile([H, B_CORE * NH], F32, tag="qtil")
            for b in range(B_CORE):
                nc.vector.tensor_scalar_mul(
                    qtil[:, NH * b:NH * (b + 1)], hm_s[:], qT_s[:, b:b + 1])

            r_ps = pss.tile([H, B_CORE * NH], F32, tag="smps")
            nc.tensor.matmul(r_ps[:], wk_s[:], qtil[:], start=True, stop=True)

            # rp_q[:, bi, 32*bi + h] = R[:, (4q+bi)*8 + h], zeros elsewhere
            rp = []
            for q in range(2):
                t = padp.tile([H, 4, H], F32, tag="pad")
                nc.sync.dma_start(
                    r(t[:].rearrange("p a c -> p (a c)")), r(zer[:]))
                for bi in range(4):
                    b = 4 * q + bi
                    nc.vector.tensor_copy(
                        r(t[:, bi, 32 * bi:32 * bi + NH]),
                        r_ps[:, NH * b:NH * (b + 1)],
                    )
                rp.append(t)

            # ---------- scores -> exp -> E^T ----------
            ets = []
            rcps = []
            for q in range(2):
                sc = psb.tile([H, N], F32, tag="bigps")
                for off, ln in SCHUNKS:
                    cs = slice(off, off + ln)
                    for bi in range(4):
                        b = 4 * q + bi
                        nc.tensor.matmul(
                            sc[:, cs], r(rp[q][:, bi, :]),
                            r(xT[:, b * N + off: b * N + off + ln]),
                            start=(bi == 0), stop=False,
                        )
                    nc.tensor.matmul(
                        sc[:, cs], r(mnegA_s[:]), r(oh_s[:, q, cs]),
                        start=False, stop=True,
                    )
                e_t = bigp.tile([H, N], F32, tag="EB")
                sums = smp.tile([H, 1], F32, tag=f"sums{q}")
                nc.scalar.activation(
                    e_t[:], sc[:], mybir.ActivationFunctionType.Exp,
                    bias=0.0, scale=1.0, accum_out=sums[:],
                )
                rcp = smp.tile([H, 1], F32, tag=f"rcp{q}")
                nc.vector.reciprocal(rcp[:], sums[:])
                rcps.append(rcp)

                et = etp.tile([NP, NJ, 32], F32, tag="et")
                for k in range(4):
                    ps = pst.tile([NP, 4, H], F32, tag="trps")
                    for i in range(4):
                        j = 4 * k + i
                        nc.tensor.transpose(
                            ps[:, i, :], e_t[:, NP * j:NP * (j + 1)],
                            id_f[:],
                        )
                    # keep only cols {32*bi + h}: E rows used by this quad
                    nc.vector.tensor_copy(
                        r(et[:, 4 * k:4 * k + 4, :]
                          .rearrange("p j (a c) -> p j a c", c=NH)),
                        ps[:].rearrange("p j (a c) -> p j a c", c=32)[
                            :, :, :, 0:NH],
                    )
                ets.append(et)

            # ---------- attnV ----------
            u_s = []
            for q in range(2):
                u_ps = pss.tile([32, 512], F32, tag="smps")
                for j in range(NJ):
                    nc.tensor.matmul(
                        u_ps[:], r(ets[q][0:NP, j, :]),
                        r(xn[q][0:NP, j, :]),
                        start=(j == 0), stop=(j == NJ - 1),
                    )
                rq_ps = pss.tile([32, 1], F32, tag="smps")
                nc.tensor.matmul(
                    rq_ps[:], sel_s[:], rcps[q][:], start=True, stop=True)
                rq_s = smp.tile([32, 1], F32, tag=f"rqs{q}")
                nc.vector.tensor_copy(rq_s[:], rq_ps[:])
                ut = smp.tile([32, 512], F32, tag=f"us{q}")
                nc.vector.tensor_scalar_mul(ut[:], u_ps[:], rq_s[:])
                u_s.append(ut)

            Ut = smp.tile([H, NH, B_CORE], F32, tag="Ut")
            for q in range(2):
                for bi in range(4):
                    b = 4 * q + bi
                    utp = pss.tile([H, 32], F32, tag="smps")
                    nc.tensor.transpose(
                        utp[:], u_s[q][0:32, H * bi:H * (bi + 1)],
                        id_f[0:32, 0:32],
                    )
                    nc.vector.tensor_copy(
                        Ut[:, :, b], utp[:, NH * bi:NH * (bi + 1)])

            v_ps = pss.tile([B_CORE, H], F32, tag="smps")
            for h in range(NH):
                nc.tensor.matmul(
                    v_ps[:, HD * h:HD * (h + 1)], Ut[:, h, :],
                    wvT_s[:, HD * h:HD * (h + 1)],
                    start=True, stop=True,
                )
            v_s = smp.tile([B_CORE, H], F32, tag="vs")
            nc.vector.tensor_copy(v_s[:], v_ps[:])

            vt_ps = pss.tile([H, B_CORE], F32, tag="smps")
            nc.tensor.transpose(vt_ps[:], v_s[:], id_f[0:B_CORE, 0:B_CORE])
            vt_s = smp.tile([H, B_CORE], F32, tag="vts")
            nc.vector.tensor_copy(vt_s[:], vt_ps[:])
            fq_ps = pss.tile([H, B_CORE], F32, tag="smps")
            nc.tensor.matmul(fq_ps[:], wcT_s[:], vt_s[:], start=True, stop=True)
            fq_s = smp.tile([H, B_CORE], F32, tag="fqs")
            nc.scalar.activation(
                fq_s[:], fq_ps[:], mybir.ActivationFunctionType.Identity,
                bias=bc_s[:, 0:1], scale=1.0,
            )

            w2_ps = pss.tile([H, B_CORE], F32, tag="smps")
            nc.tensor.matmul(w2_ps[:], wlk_s[:], fq_s[:], start=True, stop=True)
            wp = []
            for q in range(2):
                t = padp.tile([H, 4, H], F32, tag="pad")
                nc.sync.dma_start(
                    r(t[:].rearrange("p a c -> p (a c)")), r(zer[:]))
                for bi in range(4):
                    b = 4 * q + bi
                    nc.vector.tensor_copy(
                        r(t[:, bi, 32 * bi:32 * bi + 1]), w2_ps[:, b:b + 1])
                wp.append(t)

            # ---------- pointer scores + final softmax ----------
            for q in range(2):
                ps2 = psb.tile([H, N], F32, tag="bigps")
                for off, ln in SCHUNKS:
                    cs = slice(off, off + ln)
                    for bi in range(4):
                        b = 4 * q + bi
                        nc.tensor.matmul(
                            ps2[:, cs], r(wp[q][:, bi, :]),
                            r(xT[:, b * N + off: b * N + off + ln]),
                            start=(bi == 0), stop=False,
                        )
                    nc.tensor.matmul(
                        ps2[:, cs], r(mnegP_s[:]), r(oh_s[:, q, cs]),
                        start=False, stop=True,
                    )
                t_t = bigp.tile([H, N], F32, tag="EB")
                nc.scalar.activation(
                    t_t[:], ps2[:], mybir.ActivationFunctionType.Tanh)
                e2_t = bigp.tile([H, N], F32, tag="EB")
                s2 = smp.tile([H, 1], F32, tag=f"s2{q}")
                nc.scalar.activation(
                    e2_t[:], t_t[:], mybir.ActivationFunctionType.Exp,
                    bias=0.0, scale=10.0, accum_out=s2[:],
                )
                rcp2 = smp.tile([H, 1], F32, tag=f"rcp2{q}")
                nc.vector.reciprocal(rcp2[:], s2[:])
                nc.vector.tensor_scalar_mul(e2_t[:], e2_t[:], rcp2[:])
                nc.sync.dma_start(
                    probs[4 * q:4 * q + 4, :],
                    e2_t[:].rearrange("(a b) f -> a b f", b=32)[:, 0, :],
                )

    nc.compile()
    return nc


def _prep_inputs(patch_embeddings, fixed_content_cls, Wq_graph, Wq_first,
                 Wq_last, Wk, Wv, logit_Wk, Wc, bc, last_patch):
    qs = 1.0 / math.sqrt(HD)
    ls = 1.0 / math.sqrt(H)
    f32 = lambda a: np.ascontiguousarray(a, dtype=np.float32)
    shared = {
        "wqgT": f32(np.asarray(Wq_graph).T * qs),
        "wsumT": f32((np.asarray(Wq_first) + np.asarray(Wq_last)).T * qs),
        "wk": f32(Wk),
        "wvT": f32(np.asarray(Wv).T),
        "wcT": f32(np.asarray(Wc).T),
        "wlk": f32(np.asarray(logit_Wk) * ls),
        "bc": f32(np.asarray(bc)[:, None]),
        "identd": np.eye(H, dtype=np.float32),
        "seld": np.eye(H, dtype=np.float32)
            .reshape(H, 4, 32)[:, :, :8].reshape(H, 32),
        "zer": np.zeros((H, 512), np.float32),
    }
    hm = np.zeros((H, NH), np.float32)
    for h in range(NH):
        hm[HD * h:HD * (h + 1), h] = 1.0
    shared["hm"] = hm

    mnegA = np.zeros((4, H), np.float32)
    mnegP = np.zeros((4, H), np.float32)
    for bi in range(4):
        mnegA[bi, 32 * bi:32 * bi + NH] = -1e9
        mnegP[bi, 32 * bi] = -1e9
    shared["mnegA"] = mnegA
    shared["mnegP"] = mnegP

    pe = np.asarray(patch_embeddings)
    cls = np.asarray(fixed_content_cls)
    lp = np.asarray(last_patch).astype(np.int64)
    in_maps = []
    for c in range(N_CORES):
        bs = slice(B_CORE * c, B_CORE * (c + 1))
        lp_c = lp[bs]
        ohc = np.zeros((4, 2, N), np.float32)
        for b in range(B_CORE):
            ohc[b % 4, b // 4, lp_c[b]] = 1.0
        m = dict(shared)
        m["x"] = f32(pe[bs])
        m["clsT"] = f32(cls[bs, 0, :].T)
        m["oh"] = ohc
        m["roffs"] = (np.arange(B_CORE) * N + lp_c).astype(np.int32)[:, None]
        in_maps.append(m)
    return in_maps


def kernel(trace=False, **inputs):
    if "nc" not in _CACHE:
        _CACHE["nc"] = build()
    nc = _CACHE["nc"]
    in_maps = _prep_inputs(**inputs)
    res = run_bass_kernel_spmd(nc, in_maps, list(range(N_CORES)), trace=trace)
    out = np.concatenate([res.results[c]["probs"] for c in range(N_CORES)], axis=0)
    if trace:
        return out, res
    return out



# revision 13
# speedup vs baseline: 2.2362x; 2.2362x over previous
"""Trainium2 Bass kernel for nn_Decoder_74835510165950 (sparse_attention).

Single-query attention decoder over B=64, N=2000, H=128, 8 heads.
Data-parallel over 8 NeuronCores: 8 batches per core.

v2 design (vs the PE-transpose baseline):
  - Host passes X twice in fp16: xtp [c, b, n] (X^T, feeds score/pointer
    matmul rhs) and xnp [n%125, n//125, b*c] (natural, feeds attnV rhs).
    Same total DMA bytes as one fp32 X, but fully contiguous descriptors
    (the old layout produced 512B packets -> ~95 GB/s; these give 1-8KB
    runs -> near HBM rate) and ZERO on-device transposes of X.
  - All 8 batches packed into one PSUM tile per phase:
      scores  [128, n-chunk]: batch b rows 8b..8b+8 (stationary padding)
      pointer [128, n-chunk]: batch b row b
    -1e9 masking folded in as one extra matmul per chunk with a host
    one-hot rhs (-60000 fits fp16; exp/tanh saturate identically).
  - fp16 everywhere on the big streams (PSUM accumulates fp32); exp runs
    with bias=-8 so E fits fp16 (score max ~11 on this distribution);
    the softmax scale is applied to u after attnV (denominator trick).
  - DMA order interleaves xT chunks with xnat chunks so scores/attnV
    keep the PE busy while the rest of X streams in.
"""
import sys

if "/opt/trn_rl_repo" not in sys.path:
    sys.path.insert(0, "/opt/trn_rl_repo")

import math
import numpy as np

import concourse.bass as bass
import concourse.tile as tile
from concourse import bacc, mybir
from concourse.bass_utils import run_bass_kernel_spmd

F32 = mybir.dt.float32
F16 = mybir.dt.float16

N_CORES = 8
B_CORE = 8          # batches per core
N = 2000
H = 128
NH = 8              # heads
HD = 16             # head dim
NCH = 4             # score/pointer chunks
CW = 500            # chunk width (<=512 psum bank)
NJ = 16             # attnV n-chunks
NP = 125            # rows per attnV chunk
EXP_BIAS = -8.0     # uniform shift inside softmax exp (cancels in ratio)

# wpack column layout (all fp16, [128, x])
_WCOLS = {"wqgT": 0, "wsumT": 128, "wk": 256, "wvT": 384, "wcT": 512,
          "wlk": 640, "id": 768, "hm": 896, "leT": 904, "clsT": 912}
WPACK_W = 920

_CACHE = {}


def build():
    nc = bacc.Bacc("TRN2", target_bir_lowering=False, debug=False)

    xtp = nc.dram_tensor("xtp", [H, B_CORE, N], F16, kind="ExternalInput")
    xnp = nc.dram_tensor("xnp", [NP, NJ, B_CORE * H], F16, kind="ExternalInput")
    wpack = nc.dram_tensor("wpack", [H, WPACK_W], F16, kind="ExternalInput")
    small8 = nc.dram_tensor("small8", [B_CORE, N + 2 * H], F16,
                            kind="ExternalInput")
    bcd = nc.dram_tensor("bcd", [H, 1], F32, kind="ExternalInput")

    probs = nc.dram_tensor("probs", [B_CORE, N], F32, kind="ExternalOutput")

    AF = mybir.ActivationFunctionType

    with tile.TileContext(nc) as tc:
        with (
            tc.tile_pool(name="wts", bufs=1) as wts,
            tc.tile_pool(name="xt", bufs=1) as xtp_p,
            tc.tile_pool(name="xn", bufs=1) as xnp_p,
            tc.tile_pool(name="big", bufs=1) as bigp,
            tc.tile_pool(name="sm", bufs=1) as smp,
            tc.tile_pool(name="ps_big", bufs=2, space="PSUM") as psb,
            tc.tile_pool(name="ps_et", bufs=2, space="PSUM") as pse,
            tc.tile_pool(name="ps_u", bufs=1, space="PSUM") as psu,
            tc.tile_pool(name="ps_sm", bufs=2, space="PSUM") as pss,
        ):
            # ---------- DMAs ----------
            wpack_s = wts.tile([H, WPACK_W], F16, tag="wpack")
            nc.scalar.dma_start(wpack_s[:], wpack[:])
            small8_s = wts.tile([B_CORE, N + 2 * H], F16, tag="small8")
            nc.scalar.dma_start(small8_s[:], small8[:])
            bc_s = wts.tile([H, 1], F32, tag="bc")
            nc.scalar.dma_start(bc_s[:], bcd[:])

            def wcol(name, w):
                c0 = _WCOLS[name]
                return wpack_s[:, c0:c0 + w]

            wqgT16, wsumT16 = wcol("wqgT", H), wcol("wsumT", H)
            wk16 = wcol("wk", H)
            wvT16 = wcol("wvT", H)
            wcT16 = wcol("wcT", H)
            wlk16 = wcol("wlk", H)
            id16 = wcol("id", H)
            hm16 = wcol("hm", NH)
            leT16 = wcol("leT", B_CORE)
            clsT16 = wcol("clsT", B_CORE)
            oh8 = small8_s[:, 0:N]
            mnegA8 = small8_s[:, N:N + H]
            mnegP8 = small8_s[:, N + H:N + 2 * H]

            # big inputs, interleaved for pipelining
            xT = xtp_p.tile([H, B_CORE, N], F16, tag="xT")
            xn = xnp_p.tile([NP, NJ, B_CORE * H], F16, tag="xn")
            order = [("t", 0), ("t", 1), ("n", 0), ("t", 2), ("n", 1),
                     ("t", 3), ("n", 2), ("n", 3)]
            for kind, g in order:
                if kind == "t":
                    cs = slice(CW * g, CW * (g + 1))
                    nc.sync.dma_start(xT[:, :, cs], xtp[:, :, cs])
                else:
                    js = slice(4 * g, 4 * (g + 1))
                    nc.sync.dma_start(xn[:, js, :], xnp[:, js, :])

            # small PSUM tiles: one f32 [128, 128] tag (pss, 2 bufs) for
            # matmul outs, one f16 [128, 512] tag (pse, 2 bufs) shared by
            # every PE-transpose output.
            def sps():
                return pss.tile([H, H], F32, tag="smps", name="smps")

            def tps():
                return pse.tile([H, 4 * H], F16, tag="tps", name="tps")

            # ---------- Q path (runs during X DMA) ----------
            rp8 = smp.tile([H, B_CORE, H], F16, tag="rp8")
            nc.vector.memset(rp8[:], 0.0)
            wp8 = smp.tile([H, B_CORE, H], F16, tag="wp8")
            nc.vector.memset(wp8[:], 0.0)

            q_ps = sps()[:, 0:B_CORE]
            nc.tensor.matmul(q_ps, wqgT16, clsT16, start=True, stop=False)
            nc.tensor.matmul(q_ps, wsumT16, leT16, start=False, stop=True)
            q_s = smp.tile([H, B_CORE], F32, tag="q_s")
            nc.vector.tensor_copy(q_s[:], q_ps)

            qtil = smp.tile([H, B_CORE * NH], F16, tag="qtil")
            for b in range(B_CORE):
                nc.vector.tensor_scalar_mul(
                    qtil[:, NH * b:NH * (b + 1)], hm16, q_s[:, b:b + 1])
            r_ps = sps()[:, 0:B_CORE * NH]
            nc.tensor.matmul(r_ps, wk16, qtil[:], start=True, stop=True)
            for b in range(B_CORE):
                nc.vector.tensor_copy(
                    rp8[:, b, NH * b:NH * (b + 1)],
                    r_ps[:, NH * b:NH * (b + 1)])

            # ---------- scores -> exp ----------
            eb = smp.tile([H, 1], F32, tag="eb")
            nc.vector.memset(eb[:], EXP_BIAS)
            E = bigp.tile([H, N], F16, tag="E")
            sums = smp.tile([H, NCH], F32, tag="sums")
            for c in range(NCH):
                cs = slice(CW * c, CW * (c + 1))
                sc = psb.tile([H, CW], F32, tag="bigps")
                for b in range(B_CORE):
                    nc.tensor.matmul(
                        sc[:], rp8[:, b, :], xT[:, b, cs],
                        start=(b == 0), stop=False)
                nc.tensor.matmul(sc[:], mnegA8, oh8[:, cs],
                                 start=False, stop=True)
                nc.scalar.activation(
                    E[:, cs], sc[:], AF.Exp,
                    bias=eb[:, 0:1], scale=1.0, accum_out=sums[:, c:c + 1])

            stot = smp.tile([H, 1], F32, tag="stot")
            nc.vector.tensor_add(stot[:], sums[:, 0:1], sums[:, 1:2])
            nc.vector.tensor_add(stot[:], stot[:], sums[:, 2:3])
            nc.vector.tensor_add(stot[:], stot[:], sums[:, 3:4])
            rcp = smp.tile([H, 1], F32, tag="rcp")
            nc.vector.reciprocal(rcp[:], stot[:])

            # ---------- E^T (PE transposes, 125-col tiles) ----------
            et = bigp.tile([NP, NJ, H], F16, tag="et")
            for k in range(4):
                ps = tps()[0:NP, :].rearrange("p (j c) -> p j c", c=H)
                for i in range(4):
                    j = 4 * k + i
                    nc.tensor.transpose(
                        ps[:, i, :], E[:, NP * j:NP * (j + 1)], id16)
                nc.vector.tensor_copy(et[:, 4 * k:4 * k + 4, :], ps)

            # ---------- attnV: u[bh, (b', c)] = sum_n E^T X ----------
            u_ps = [psu.tile([H, 4 * H], F32, tag=f"ups{h}", name=f"ups{h}")
                    for h in range(2)]
            for j in range(NJ):
                for h in range(2):
                    nc.tensor.matmul(
                        u_ps[h][:], et[:, j, :],
                        xn[:, j, 4 * H * h:4 * H * (h + 1)],
                        start=(j == 0), stop=(j == NJ - 1))
            us = smp.tile([H, 2, 4 * H], F16, tag="us")
            for h in range(2):
                nc.vector.tensor_scalar_mul(us[:, h, :], u_ps[h][:], rcp[:, 0:1])

            # ---------- uT, v, fq, w2 ----------
            uT = smp.tile([H, B_CORE * NH], F16, tag="uT")
            for b in range(B_CORE):
                o = H * (b % 4)
                ps = tps()[:, 0:H]
                nc.tensor.transpose(ps, us[:, b // 4, o:o + H], id16)
                nc.vector.tensor_copy(
                    uT[:, NH * b:NH * (b + 1)], ps[:, NH * b:NH * (b + 1)])

            v_ps = sps()[0:B_CORE, :]
            for h in range(NH):
                nc.tensor.matmul(
                    v_ps[:, HD * h:HD * (h + 1)],
                    uT[:].rearrange("p (b h) -> p h b", h=NH)[:, h, :],
                    wvT16[:, HD * h:HD * (h + 1)],
                    start=True, stop=True)
            v_s = smp.tile([B_CORE, H], F16, tag="v_s")
            nc.vector.tensor_copy(v_s[:], v_ps)

            vt_ps = tps()[:, 0:B_CORE]
            nc.tensor.transpose(vt_ps, v_s[:], id16[0:B_CORE, 0:B_CORE])
            vT = smp.tile([H, B_CORE], F16, tag="vT")
            nc.vector.tensor_copy(vT[:], vt_ps)

            fq_ps = sps()[:, 0:B_CORE]
            nc.tensor.matmul(fq_ps, wcT16, vT[:], start=True, stop=True)
            fq_s = smp.tile([H, B_CORE], F16, tag="fq_s")
            nc.scalar.activation(
                fq_s[:], fq_ps, AF.Identity, bias=bc_s[:, 0:1], scale=1.0)

            w2_ps = sps()[:, 0:B_CORE]
            nc.tensor.matmul(w2_ps, wlk16, fq_s[:], start=True, stop=True)
            for b in range(B_CORE):
                nc.vector.tensor_copy(
                    wp8[:, b, b:b + 1], w2_ps[:, b:b + 1])

            # ---------- pointer scores -> tanh -> exp -> norm ----------
            tp = bigp.tile([B_CORE, N], F16, tag="tp")
            e2 = bigp.tile([B_CORE, N], F32, tag="e2")
            s2s = smp.tile([B_CORE, NCH], F32, tag="s2s")
            for c in range(NCH):
                cs = slice(CW * c, CW * (c + 1))
                pc = psb.tile([H, CW], F32, tag="bigps")
                for b in range(B_CORE):
                    nc.tensor.matmul(
                        pc[:], wp8[:, b, :], xT[:, b, cs],
                        start=(b == 0), stop=False)
                nc.tensor.matmul(pc[:], mnegP8, oh8[:, cs],
                                 start=False, stop=True)
                nc.scalar.activation(tp[:, cs], pc[0:B_CORE, :], AF.Tanh)
                nc.scalar.activation(
                    e2[:, cs], tp[:, cs], AF.Exp,
                    bias=0.0, scale=10.0, accum_out=s2s[:, c:c + 1])

            s2tot = smp.tile([B_CORE, 1], F32, tag="s2tot")
            nc.vector.tensor_add(s2tot[:], s2s[:, 0:1], s2s[:, 1:2])
            nc.vector.tensor_add(s2tot[:], s2tot[:], s2s[:, 2:3])
            nc.vector.tensor_add(s2tot[:], s2tot[:], s2s[:, 3:4])
            rcp2 = smp.tile([B_CORE, 1], F32, tag="rcp2")
            nc.vector.reciprocal(rcp2[:], s2tot[:])
            nc.vector.tensor_scalar_mul(e2[:], e2[:], rcp2[:, 0:1])

            nc.sync.dma_start(probs[:], e2[:])

    nc.compile()
    return nc


def _prep_inputs(patch_embeddings, fixed_content_cls, Wq_graph, Wq_first,
                 Wq_last, Wk, Wv, logit_Wk, Wc, bc, last_patch):
    qs = 1.0 / math.sqrt(HD)
    ls = 1.0 / math.sqrt(H)
    f16 = lambda a: np.ascontiguousarray(a, dtype=np.float16)
    f32 = lambda a: np.ascontiguousarray(a, dtype=np.float32)

    hm = np.zeros((H, NH), np.float32)
    for h in range(NH):
        hm[HD * h:HD * (h + 1), h] = qs

    pe = np.asarray(patch_embeddings, dtype=np.float32)
    cls = np.asarray(fixed_content_cls, dtype=np.float32)
    lp = np.asarray(last_patch).astype(np.int64)

    wpack_base = np.zeros((H, WPACK_W), np.float16)
    wpack_base[:, 0:128] = f16(np.asarray(Wq_graph).T)
    wpack_base[:, 128:256] = f16(np.asarray(Wq_first) + np.asarray(Wq_last)).T
    wpack_base[:, 256:384] = f16(Wk)
    wpack_base[:, 384:512] = f16(np.asarray(Wv).T)
    wpack_base[:, 512:640] = f16(np.asarray(Wc).T)
    wpack_base[:, 640:768] = f16(np.asarray(logit_Wk) * ls)
    wpack_base[:, 768:896] = np.eye(H, dtype=np.float16)
    wpack_base[:, 896:904] = f16(hm)

    bc_arr = f32(np.asarray(bc)[:, None])

    in_maps = []
    for c in range(N_CORES):
        bs = slice(B_CORE * c, B_CORE * (c + 1))
        pec = pe[bs]                        # (8, 2000, 128)
        lp_c = lp[bs]
        wpack = wpack_base.copy()
        wpack[:, 904:912] = f16(pec[np.arange(B_CORE), lp_c].T)   # leT
        wpack[:, 912:920] = f16(cls[bs, 0, :].T)                   # clsT

        small8 = np.zeros((B_CORE, N + 2 * H), np.float16)
        small8[np.arange(B_CORE), lp_c] = 1.0                      # oh8
        for b in range(B_CORE):
            small8[b, N + NH * b:N + NH * (b + 1)] = -60000.0      # mnegA8
            small8[b, N + H + b] = -60000.0                        # mnegP8

        m = {
            "xtp": f16(pec.transpose(2, 0, 1)),                    # (128,8,2000)
            "xnp": f16(pec.reshape(B_CORE, NJ, NP, H)
                       .transpose(2, 1, 0, 3).reshape(NP, NJ, B_CORE * H)),
            "wpack": wpack,
            "small8": small8,
            "bcd": bc_arr,
        }
        in_maps.append(m)
    return in_maps


def kernel(trace=False, **inputs):
    if "nc" not in _CACHE:
        _CACHE["nc"] = build()
    nc = _CACHE["nc"]
    in_maps = _prep_inputs(**inputs)
    res = run_bass_kernel_spmd(nc, in_maps, list(range(N_CORES)), trace=trace)
    out = np.concatenate(
        [res.results[c]["probs"].astype(np.float32) for c in range(N_CORES)],
        axis=0)
    if trace:
        return out, res
    return out


# revision 16
# speedup vs baseline: 2.5310x; 1.1318x over previous
"""Trainium2 Bass kernel for nn_Decoder_74835510165950 (sparse_attention).

Single-query attention decoder over B=64, N=2000, H=128, 8 heads.
Data-parallel over 8 NeuronCores: 8 batches per core.

v3 design:
  - DMA traffic halved vs v2: ONLY X^T fp16 is loaded (4.2MB/core, padded
    to N=2048, chunk-major layout -> 8KB contiguous runs per partition).
    X-natural (for attnV) is rebuilt on device with regular-matmul
    transposes (lhsT = xT 128-col tile, rhs = identity), which count as
    real PE activity (HAM stays warm) and get fp16 fast-weight-load.
  - All 8 batches packed per PSUM tile: scores rows 8b..8b+8, pointer
    row b. Masking (-60000, fits fp16) via one extra matmul per chunk
    with a host one-hot rhs; a 9th row masks the 48 pad columns.
  - fp16 streams everywhere (PSUM fp32); softmax exp biased by -8 so E
    fits fp16 (score max ~11); 1/sum applied to u after attnV.
  - Fully chunk-pipelined: per 512-col chunk: DMA -> scores -> exp ->
    xnat rebuild + E^T -> attnV, so compute hides under the DMA stream;
    only the small chain + pointer phase trail the last chunk.
"""
import sys

if "/opt/trn_rl_repo" not in sys.path:
    sys.path.insert(0, "/opt/trn_rl_repo")

import math
import numpy as np

import concourse.bass as bass
import concourse.tile as tile
from concourse import bacc, mybir
from concourse.bass_utils import run_bass_kernel_spmd

F32 = mybir.dt.float32
F16 = mybir.dt.float16

N_CORES = 8
B_CORE = 8          # batches per core
N = 2000
N2 = 2048           # padded
H = 128
NH = 8              # heads
HD = 16             # head dim
NCH = 4             # chunks
CW = 512            # chunk width (= one psum bank of fp32)
NJ = 16             # 128-col n-tiles (N2 / 128)
EXP_BIAS = -8.0     # uniform shift inside softmax exp (cancels in ratio)
MNEG = -60000.0

# wpack column layout (all fp16, [128, x])
_WCOLS = {"wqgT": 0, "wsumT": 128, "wk": 256, "wvT": 384, "wcT": 512,
          "wlk": 640, "id": 768, "hm": 896, "leT": 904, "clsT": 912}
WPACK_W = 920
SM_W = N2 + 2 * H   # small9 row width

_CACHE = {}


def build():
    nc = bacc.Bacc("TRN2", target_bir_lowering=False, debug=False)

    xtp = nc.dram_tensor("xtp", [H, NCH, B_CORE, CW], F16, kind="ExternalInput")
    wpack = nc.dram_tensor("wpack", [H, WPACK_W], F16, kind="ExternalInput")
    small9 = nc.dram_tensor("small9", [B_CORE + 1, SM_W], F16,
                            kind="ExternalInput")
    bcd = nc.dram_tensor("bcd", [H, 1], F32, kind="ExternalInput")

    probs = nc.dram_tensor("probs", [B_CORE, N], F32, kind="ExternalOutput")

    AF = mybir.ActivationFunctionType

    with tile.TileContext(nc) as tc:
        with (
            tc.tile_pool(name="wts", bufs=1) as wts,
            tc.tile_pool(name="xt", bufs=1) as xtp_p,
            tc.tile_pool(name="xn", bufs=1) as xnp_p,
            tc.tile_pool(name="big", bufs=1) as bigp,
            tc.tile_pool(name="sm", bufs=1) as smp,
            tc.tile_pool(name="ps_sc", bufs=1, space="PSUM") as psc,
            tc.tile_pool(name="ps_big", bufs=2, space="PSUM") as psb,
            tc.tile_pool(name="ps_et", bufs=2, space="PSUM") as pse,
            tc.tile_pool(name="ps_u", bufs=1, space="PSUM") as psu,
            tc.tile_pool(name="ps_sm", bufs=1, space="PSUM") as pss,
        ):
            # ---------- small DMAs (scalar queue, in parallel with X) ----
            wpack_s = wts.tile([H, WPACK_W], F16, tag="wpack")
            nc.scalar.dma_start(wpack_s[:], wpack[:])
            small9_s = wts.tile([B_CORE + 1, SM_W], F16, tag="small9")
            nc.scalar.dma_start(small9_s[:], small9[:])
            bc_s = wts.tile([H, 1], F32, tag="bc")
            nc.scalar.dma_start(bc_s[:], bcd[:])

            def wcol(name, w):
                c0 = _WCOLS[name]
                return wpack_s[:, c0:c0 + w]

            wqgT16, wsumT16 = wcol("wqgT", H), wcol("wsumT", H)
            wk16 = wcol("wk", H)
            wvT16 = wcol("wvT", H)
            wcT16 = wcol("wcT", H)
            wlk16 = wcol("wlk", H)
            id16 = wcol("id", H)
            hm16 = wcol("hm", NH)
            leT16 = wcol("leT", B_CORE)
            clsT16 = wcol("clsT", B_CORE)
            oh9 = small9_s[:, 0:N2]
            mnegA9 = small9_s[:, N2:N2 + H]
            mnegP9 = small9_s[:, N2 + H:N2 + 2 * H]

            # ---------- X^T chunk DMAs (sync queue) ----------
            xT = xtp_p.tile([H, NCH, B_CORE, CW], F16, tag="xT")
            for c in range(NCH):
                nc.sync.dma_start(xT[:, c, :, :], xtp[:, c, :, :])

            def sps():
                return pss.tile([H, H], F32, tag="smps", name="smps")

            def tps():
                return pse.tile([H, 4 * H], F16, tag="tps", name="tps")

            # ---------- Q path (runs during X DMA) ----------
            rp8 = smp.tile([H, B_CORE, H], F16, tag="rp8")
            nc.vector.memset(rp8[:], 0.0)
            wp8 = smp.tile([H, B_CORE, H], F16, tag="wp8")
            nc.vector.memset(wp8[:], 0.0)
            eb = smp.tile([H, 1], F32, tag="eb")
            nc.vector.memset(eb[:], EXP_BIAS)

            q_ps = sps()[:, 0:B_CORE]
            nc.tensor.matmul(q_ps, wqgT16, clsT16, start=True, stop=False)
            nc.tensor.matmul(q_ps, wsumT16, leT16, start=False, stop=True)
            q_s = smp.tile([H, B_CORE], F32, tag="q_s")
            nc.vector.tensor_copy(q_s[:], q_ps)

            qtil = smp.tile([H, B_CORE * NH], F16, tag="qtil")
            for b in range(B_CORE):
                nc.vector.tensor_scalar_mul(
                    qtil[:, NH * b:NH * (b + 1)], hm16, q_s[:, b:b + 1])
            r_ps = sps()[:, 0:B_CORE * NH]
            nc.tensor.matmul(r_ps, wk16, qtil[:], start=True, stop=True)
            for b in range(B_CORE):
                nc.vector.tensor_copy(
                    rp8[:, b, NH * b:NH * (b + 1)],
                    r_ps[:, NH * b:NH * (b + 1)])

            # ---------- main chunk pipeline ----------
            E = bigp.tile([H, N2], F16, tag="E")
            sums = smp.tile([H, NCH], F32, tag="sums")
            et = bigp.tile([H, NJ, H], F16, tag="et")
            xn = xnp_p.tile([H, NJ, B_CORE, H], F16, tag="xn")
            u_ps = [psu.tile([H, 4 * H], F32, tag=f"ups{g}", name=f"ups{g}")
                    for g in range(2)]

            for c in range(NCH):
                xTc = xT[:, c, :, :]
                cs = slice(CW * c, CW * (c + 1))
                # scores
                sc = psc.tile([H, CW], F32, tag="scps", name="sc")
                for b in range(B_CORE):
                    nc.tensor.matmul(sc[:], rp8[:, b, :], xTc[:, b, :],
                                     start=(b == 0), stop=False)
                nc.tensor.matmul(sc[:], mnegA9, oh9[:, cs],
                                 start=False, stop=True)
                nc.scalar.activation(
                    E[:, cs], sc[:], AF.Exp,
                    bias=eb[:, 0:1], scale=1.0, accum_out=sums[:, c:c + 1])

                # rebuild X-natural for this chunk (regular-matmul transposes)
                for i in range(4):
                    j = 4 * c + i
                    for bg in (0, 4):
                        xb = psb.tile([H, 4 * H], F32, tag="bigps", name="xb")
                        for k in range(4):
                            b = bg + k
                            nc.tensor.matmul(
                                xb[:, H * k:H * (k + 1)],
                                xTc[:, b, H * i:H * (i + 1)], id16,
                                start=True, stop=True)
                        nc.vector.tensor_copy(
                            xn[:, j, bg:bg + 4, :]
                            .rearrange("p a c -> p (a c)"), xb[:])

                # E^T for this chunk
                ep = tps()
                for i in range(4):
                    j = 4 * c + i
                    nc.tensor.transpose(
                        ep[:, H * i:H * (i + 1)], E[:, H * j:H * (j + 1)],
                        id16)
                nc.vector.tensor_copy(
                    et[:, 4 * c:4 * c + 4, :]
                    .rearrange("p a c -> p (a c)"), ep[:])

                # attnV partial accumulation for this chunk
                for i in range(4):
                    j = 4 * c + i
                    for g in range(2):
                        nc.tensor.matmul(
                            u_ps[g][:], et[:, j, :],
                            xn[:, j, 4 * g:4 * g + 4, :]
                            .rearrange("p a c -> p (a c)"),
                            start=(j == 0), stop=(j == NJ - 1))

            stot = smp.tile([H, 1], F32, tag="stot")
            nc.vector.tensor_add(stot[:], sums[:, 0:1], sums[:, 1:2])
            nc.vector.tensor_add(stot[:], stot[:], sums[:, 2:3])
            nc.vector.tensor_add(stot[:], stot[:], sums[:, 3:4])
            rcp = smp.tile([H, 1], F32, tag="rcp")
            nc.vector.reciprocal(rcp[:], stot[:])

            us = smp.tile([H, 2, 4 * H], F16, tag="us")
            for g in range(2):
                nc.vector.tensor_scalar_mul(us[:, g, :], u_ps[g][:],
                                            rcp[:, 0:1])

            # ---------- uT, v, fq, w2 ----------
            uT = smp.tile([H, B_CORE * NH], F16, tag="uT")
            for b in range(B_CORE):
                o = H * (b % 4)
                ps = tps()[:, 0:H]
                nc.tensor.transpose(ps, us[:, b // 4, o:o + H], id16)
                nc.vector.tensor_copy(
                    uT[:, NH * b:NH * (b + 1)], ps[:, NH * b:NH * (b + 1)])

            v_ps = sps()[0:B_CORE, :]
            for h in range(NH):
                nc.tensor.matmul(
                    v_ps[:, HD * h:HD * (h + 1)],
                    uT[:].rearrange("p (b h) -> p h b", h=NH)[:, h, :],
                    wvT16[:, HD * h:HD * (h + 1)],
                    start=True, stop=True)
            v_s = smp.tile([B_CORE, H], F16, tag="v_s")
            nc.vector.tensor_copy(v_s[:], v_ps)

            vt_ps = tps()[:, 0:B_CORE]
            nc.tensor.transpose(vt_ps, v_s[:], id16[0:B_CORE, 0:B_CORE])
            vT = smp.tile([H, B_CORE], F16, tag="vT")
            nc.vector.tensor_copy(vT[:], vt_ps)

            fq_ps = sps()[:, 0:B_CORE]
            nc.tensor.matmul(fq_ps, wcT16, vT[:], start=True, stop=True)
            fq_s = smp.tile([H, B_CORE], F16, tag="fq_s")
            nc.scalar.activation(
                fq_s[:], fq_ps, AF.Identity, bias=bc_s[:, 0:1], scale=1.0)

            w2_ps = sps()[:, 0:B_CORE]
            nc.tensor.matmul(w2_ps, wlk16, fq_s[:], start=True, stop=True)
            for b in range(B_CORE):
                nc.vector.tensor_copy(wp8[:, b, b:b + 1], w2_ps[:, b:b + 1])

            # ---------- pointer scores -> tanh -> exp -> norm ----------
            tp = bigp.tile([B_CORE, N2], F16, tag="tp")
            e2 = bigp.tile([B_CORE, N2], F32, tag="e2")
            s2s = smp.tile([B_CORE, NCH], F32, tag="s2s")
            for c in range(NCH):
                xTc = xT[:, c, :, :]
                cs = slice(CW * c, CW * (c + 1))
                pc = psb.tile([H, CW], F32, tag="bigps", name="pc")
                for b in range(B_CORE):
                    nc.tensor.matmul(pc[:], wp8[:, b, :], xTc[:, b, :],
                                     start=(b == 0), stop=False)
                nc.tensor.matmul(pc[:], mnegP9, oh9[:, cs],
                                 start=False, stop=True)
                nc.scalar.activation(tp[:, cs], pc[0:B_CORE, :], AF.Tanh)
                nc.scalar.activation(
                    e2[:, cs], tp[:, cs], AF.Exp,
                    bias=0.0, scale=10.0, accum_out=s2s[:, c:c + 1])

            s2tot = smp.tile([B_CORE, 1], F32, tag="s2tot")
            nc.vector.tensor_add(s2tot[:], s2s[:, 0:1], s2s[:, 1:2])
            nc.vector.tensor_add(s2tot[:], s2tot[:], s2s[:, 2:3])
            nc.vector.tensor_add(s2tot[:], s2tot[:], s2s[:, 3:4])
            rcp2 = smp.tile([B_CORE, 1], F32, tag="rcp2")
            nc.vector.reciprocal(rcp2[:], s2tot[:])
            nc.vector.tensor_scalar_mul(e2[:, 0:N], e2[:, 0:N], rcp2[:, 0:1])

            nc.sync.dma_start(probs[:], e2[:, 0:N])

    nc.compile()
    return nc


def _prep_inputs(patch_embeddings, fixed_content_cls, Wq_graph, Wq_first,
                 Wq_last, Wk, Wv, logit_Wk, Wc, bc, last_patch):
    qs = 1.0 / math.sqrt(HD)
    ls = 1.0 / math.sqrt(H)
    f16 = lambda a: np.ascontiguousarray(a, dtype=np.float16)
    f32 = lambda a: np.ascontiguousarray(a, dtype=np.float32)

    hm = np.zeros((H, NH), np.float32)
    for h in range(NH):
        hm[HD * h:HD * (h + 1), h] = qs

    pe = np.asarray(patch_embeddings, dtype=np.float32)
    cls = np.asarray(fixed_content_cls, dtype=np.float32)
    lp = np.asarray(last_patch).astype(np.int64)

    wpack_base = np.zeros((H, WPACK_W), np.float16)
    wpack_base[:, 0:128] = f16(np.asarray(Wq_graph).T)
    wpack_base[:, 128:256] = f16(np.asarray(Wq_first) + np.asarray(Wq_last)).T
    wpack_base[:, 256:384] = f16(Wk)
    wpack_base[:, 384:512] = f16(np.asarray(Wv).T)
    wpack_base[:, 512:640] = f16(np.asarray(Wc).T)
    wpack_base[:, 640:768] = f16(np.asarray(logit_Wk) * ls)
    wpack_base[:, 768:896] = np.eye(H, dtype=np.float16)
    wpack_base[:, 896:904] = f16(hm)

    bc_arr = f32(np.asarray(bc)[:, None])

    in_maps = []
    for c in range(N_CORES):
        bs = slice(B_CORE * c, B_CORE * (c + 1))
        pec = pe[bs]                        # (8, 2000, 128)
        lp_c = lp[bs]
        wpack = wpack_base.copy()
        wpack[:, 904:912] = f16(pec[np.arange(B_CORE), lp_c].T)   # leT
        wpack[:, 912:920] = f16(cls[bs, 0, :].T)                   # clsT

        pad = np.zeros((B_CORE, N2, H), np.float16)
        pad[:, :N, :] = pec
        # (b, chunk, n, c) -> (c, chunk, b, n)
        xtp = np.ascontiguousarray(
            pad.reshape(B_CORE, NCH, CW, H).transpose(3, 1, 0, 2))

        small9 = np.zeros((B_CORE + 1, SM_W), np.float16)
        small9[np.arange(B_CORE), lp_c] = 1.0                      # one-hot
        small9[B_CORE, N:N2] = 1.0                                 # pad cols
        for b in range(B_CORE):
            small9[b, N2 + NH * b:N2 + NH * (b + 1)] = MNEG        # mnegA9
            small9[b, N2 + H + b] = MNEG                           # mnegP9
        small9[B_CORE, N2:N2 + 2 * H] = MNEG                       # pad row

        m = {"xtp": xtp, "wpack": wpack, "small9": small9, "bcd": bc_arr}
        in_maps.append(m)
    return in_maps


def kernel(trace=False, **inputs):
    if "nc" not in _CACHE:
        _CACHE["nc"] = build()
    nc = _CACHE["nc"]
    in_maps = _prep_inputs(**inputs)
    res = run_bass_kernel_spmd(nc, in_maps, list(range(N_CORES)), trace=trace)
    out = np.concatenate(
        [res.results[c]["probs"].astype(np.float32) for c in range(N_CORES)],
        axis=0)
    if trace:
        return out, res
    return out


# revision 18
# speedup vs baseline: 2.9304x; 1.1578x over previous
"""Trainium2 Bass kernel for nn_Decoder_74835510165950 (sparse_attention).

Single-query attention decoder over B=64, N=2000, H=128, 8 heads.
Data-parallel over 8 NeuronCores: 8 batches per core.

v4 design:
  - Only X^T fp16 is DMA'd (4.2MB/core, N padded to 2048, chunk-major
    -> 8KB contiguous runs, ~350GB/s). Small tensors go FIRST on the
    same queue so the Q-path isn't starved behind the bulk load.
  - X-natural (for attnV) is rebuilt on device: 8 PE transposes into one
    fp16 [128,1024] PSUM bank per 128-col n-tile, then a single copy,
    alternating Scalar/Vector engines so neither serializes the PE.
  - All 8 batches packed per PSUM tile (scores rows 8b..8b+8, pointer
    row b); -60000 masking via one extra matmul per chunk with a host
    one-hot rhs; a 9th row masks the 48 pad columns.
  - fp16 streams everywhere (PSUM fp32); softmax exp biased by -8 so E
    fits fp16; 1/sum applied to u after attnV; logit_Wk^T@Wc and
    logit_Wk^T@bc folded on the host so the output head is one matmul.
  - Fully chunk-pipelined: DMA -> scores -> exp -> xnat rebuild + E^T ->
    attnV per 512-col chunk; only the small chain + pointer phase trail
    the last chunk.
"""
import sys

if "/opt/trn_rl_repo" not in sys.path:
    sys.path.insert(0, "/opt/trn_rl_repo")

import math
import numpy as np

import concourse.bass as bass
import concourse.tile as tile
from concourse import bacc, mybir
from concourse.bass_utils import run_bass_kernel_spmd

F32 = mybir.dt.float32
F16 = mybir.dt.float16

N_CORES = 8
B_CORE = 8          # batches per core
N = 2000
N2 = 2048           # padded
H = 128
NH = 8              # heads
HD = 16             # head dim
NCH = 4             # chunks
CW = 512            # chunk width (= one psum bank of fp32)
NJ = 16             # 128-col n-tiles (N2 / 128)
EXP_BIAS = -8.0     # uniform shift inside softmax exp (cancels in ratio)
MNEG = -60000.0

# wpack column layout (all fp16, [128, x])
_WCOLS = {"wqgT": 0, "wsumT": 128, "wk": 256, "wvT": 384, "wlc": 512,
          "id": 640, "hm": 768, "leT": 776, "clsT": 784}
WPACK_W = 792
SM_W = N2 + 2 * H   # small9 row width

_CACHE = {}


def build():
    nc = bacc.Bacc("TRN2", target_bir_lowering=False, debug=False)

    xtp = nc.dram_tensor("xtp", [H, NCH, B_CORE, CW], F16, kind="ExternalInput")
    wpack = nc.dram_tensor("wpack", [H, WPACK_W], F16, kind="ExternalInput")
    small9 = nc.dram_tensor("small9", [B_CORE + 1, SM_W], F16,
                            kind="ExternalInput")
    blcd = nc.dram_tensor("blcd", [H, 1], F32, kind="ExternalInput")

    probs = nc.dram_tensor("probs", [B_CORE, N], F32, kind="ExternalOutput")

    AF = mybir.ActivationFunctionType

    with tile.TileContext(nc) as tc:
        with (
            tc.tile_pool(name="wts", bufs=1) as wts,
            tc.tile_pool(name="xt", bufs=1) as xtp_p,
            tc.tile_pool(name="xn", bufs=1) as xnp_p,
            tc.tile_pool(name="big", bufs=1) as bigp,
            tc.tile_pool(name="sm", bufs=1) as smp,
            tc.tile_pool(name="ps_sc", bufs=2, space="PSUM") as psc,
            tc.tile_pool(name="ps_tr", bufs=3, space="PSUM") as pst,
            tc.tile_pool(name="ps_u", bufs=1, space="PSUM") as psu,
            tc.tile_pool(name="ps_sm", bufs=1, space="PSUM") as pss,
        ):
            # ---------- DMAs: smalls first, then X^T chunks ----------
            wpack_s = wts.tile([H, WPACK_W], F16, tag="wpack")
            nc.sync.dma_start(wpack_s[:], wpack[:])
            small9_s = wts.tile([B_CORE + 1, SM_W], F16, tag="small9")
            nc.sync.dma_start(small9_s[:], small9[:])
            blc_s = wts.tile([H, 1], F32, tag="blc")
            nc.sync.dma_start(blc_s[:], blcd[:])

            def wcol(name, w):
                c0 = _WCOLS[name]
                return wpack_s[:, c0:c0 + w]

            wqgT16, wsumT16 = wcol("wqgT", H), wcol("wsumT", H)
            wk16 = wcol("wk", H)
            wvT16 = wcol("wvT", H)
            wlc16 = wcol("wlc", H)
            id16 = wcol("id", H)
            hm16 = wcol("hm", NH)
            leT16 = wcol("leT", B_CORE)
            clsT16 = wcol("clsT", B_CORE)
            oh9 = small9_s[:, 0:N2]
            mnegA9 = small9_s[:, N2:N2 + H]
            mnegP9 = small9_s[:, N2 + H:N2 + 2 * H]

            xT = xtp_p.tile([H, NCH, B_CORE, CW], F16, tag="xT")
            for c in range(NCH):
                nc.sync.dma_start(xT[:, c, :, :], xtp[:, c, :, :])

            def sps():
                return pss.tile([H, H], F32, tag="smps", name="smps")

            def tps():
                return pst.tile([H, 8 * H], F16, tag="tps", name="tps")

            # ---------- Q path (runs during X DMA) ----------
            rp8 = smp.tile([H, B_CORE, H], F16, tag="rp8")
            nc.gpsimd.memset(rp8[:], 0.0)
            wp8 = smp.tile([H, B_CORE, H], F16, tag="wp8")
            nc.gpsimd.memset(wp8[:], 0.0)
            eb = smp.tile([H, 1], F32, tag="eb")
            nc.vector.memset(eb[:], EXP_BIAS)
            z16 = smp.tile([H, 1], F16, tag="z16")
            nc.vector.memset(z16[:], 0.0)

            q_ps = sps()[:, 0:B_CORE]
            nc.tensor.matmul(q_ps, wqgT16, clsT16, start=True, stop=False)
            nc.tensor.matmul(q_ps, wsumT16, leT16, start=False, stop=True)
            q_s = smp.tile([H, B_CORE], F32, tag="q_s")
            nc.vector.tensor_copy(q_s[:], q_ps)

            qtil = smp.tile([H, B_CORE * NH], F16, tag="qtil")
            for b in range(B_CORE):
                if b % 2 == 0:
                    nc.vector.tensor_scalar_mul(
                        qtil[:, NH * b:NH * (b + 1)], hm16, q_s[:, b:b + 1])
                else:
                    nc.scalar.activation(
                        qtil[:, NH * b:NH * (b + 1)], hm16, AF.Identity,
                        bias=z16[:, 0:1], scale=q_s[:, b:b + 1])
            r_ps = sps()[:, 0:B_CORE * NH]
            nc.tensor.matmul(r_ps, wk16, qtil[:], start=True, stop=True)
            for b in range(B_CORE):
                if b % 2 == 0:
                    nc.vector.tensor_copy(
                        rp8[:, b, NH * b:NH * (b + 1)],
                        r_ps[:, NH * b:NH * (b + 1)])
                else:
                    nc.scalar.activation(
                        rp8[:, b, NH * b:NH * (b + 1)],
                        r_ps[:, NH * b:NH * (b + 1)], AF.Identity,
                        bias=0.0, scale=1.0)

            # ---------- main chunk pipeline ----------
            E = bigp.tile([H, N2], F16, tag="E")
            sums = smp.tile([H, NCH], F32, tag="sums")
            et = bigp.tile([H, NJ, H], F16, tag="et")
            xn = xnp_p.tile([H, NJ, B_CORE, H], F16, tag="xn")
            u_ps = [psu.tile([H, 4 * H], F32, tag=f"ups{g}", name=f"ups{g}")
                    for g in range(2)]

            for c in range(NCH):
                xTc = xT[:, c, :, :]
                cs = slice(CW * c, CW * (c + 1))
                # scores
                sc = psc.tile([H, CW], F32, tag="scps", name="sc")
                for b in range(B_CORE):
                    nc.tensor.matmul(sc[:], rp8[:, b, :], xTc[:, b, :],
                                     start=(b == 0), stop=False)
                nc.tensor.matmul(sc[:], mnegA9, oh9[:, cs],
                                 start=False, stop=True)
                nc.scalar.activation(
                    E[:, cs], sc[:], AF.Exp,
                    bias=eb[:, 0:1], scale=1.0, accum_out=sums[:, c:c + 1])

                # rebuild X-natural for this chunk (PE transposes, f16 PSUM)
                for i in range(4):
                    j = 4 * c + i
                    xb = tps()
                    for b in range(B_CORE):
                        nc.tensor.transpose(
                            xb[:, H * b:H * (b + 1)],
                            xTc[:, b, H * i:H * (i + 1)], id16)
                    dst = xn[:, j, :, :].rearrange("p a c -> p (a c)")
                    if i % 2 == 0:
                        nc.vector.tensor_copy(dst, xb[:])
                    else:
                        nc.scalar.activation(dst, xb[:], AF.Identity,
                                             bias=z16[:, 0:1], scale=1.0)

                # E^T for this chunk
                ep = tps()[:, 0:4 * H]
                for i in range(4):
                    j = 4 * c + i
                    nc.tensor.transpose(
                        ep[:, H * i:H * (i + 1)], E[:, H * j:H * (j + 1)],
                        id16)
                nc.vector.tensor_copy(
                    et[:, 4 * c:4 * c + 4, :]
                    .rearrange("p a c -> p (a c)"), ep)

                # attnV partial accumulation for this chunk
                for i in range(4):
                    j = 4 * c + i
                    for g in range(2):
                        nc.tensor.matmul(
                            u_ps[g][:], et[:, j, :],
                            xn[:, j, 4 * g:4 * g + 4, :]
                            .rearrange("p a c -> p (a c)"),
                            start=(j == 0), stop=(j == NJ - 1))

            s01 = smp.tile([H, 2], F32, tag="s01")
            nc.vector.tensor_add(s01[:, 0:1], sums[:, 0:1], sums[:, 1:2])
            nc.vector.tensor_add(s01[:, 1:2], sums[:, 2:3], sums[:, 3:4])
            stot = smp.tile([H, 1], F32, tag="stot")
            nc.vector.tensor_add(stot[:], s01[:, 0:1], s01[:, 1:2])
            rcp = smp.tile([H, 1], F32, tag="rcp")
            nc.vector.reciprocal(rcp[:], stot[:])

            us = smp.tile([H, 2, 4 * H], F16, tag="us")
            for g in range(2):
                nc.vector.tensor_scalar_mul(us[:, g, :], u_ps[g][:],
                                            rcp[:, 0:1])

            # ---------- uT, v, w2 ----------
            uT = smp.tile([H, B_CORE * NH], F16, tag="uT")
            for b in range(B_CORE):
                o = H * (b % 4)
                ps = tps()[:, 0:H]
                nc.tensor.transpose(ps, us[:, b // 4, o:o + H], id16)
                nc.vector.tensor_copy(
                    uT[:, NH * b:NH * (b + 1)], ps[:, NH * b:NH * (b + 1)])

            v_ps = sps()[0:B_CORE, :]
            for h in range(NH):
                nc.tensor.matmul(
                    v_ps[:, HD * h:HD * (h + 1)],
                    uT[:].rearrange("p (b h) -> p h b", h=NH)[:, h, :],
                    wvT16[:, HD * h:HD * (h + 1)],
                    start=True, stop=True)
            v_s = smp.tile([B_CORE, H], F16, tag="v_s")
            nc.vector.tensor_copy(v_s[:], v_ps)

            vt_ps = tps()[:, 0:B_CORE]
            nc.tensor.transpose(vt_ps, v_s[:], id16[0:B_CORE, 0:B_CORE])
            vT = smp.tile([H, B_CORE], F16, tag="vT")
            nc.vector.tensor_copy(vT[:], vt_ps)

            # w2 = (ls * logit_Wk^T Wc) v + ls * logit_Wk^T bc  (host-folded)
            w2_ps = sps()[:, 0:B_CORE]
            nc.tensor.matmul(w2_ps, wlc16, vT[:], start=True, stop=True)
            w2_s = smp.tile([H, B_CORE], F16, tag="w2_s")
            nc.scalar.activation(
                w2_s[:], w2_ps, AF.Identity, bias=blc_s[:, 0:1], scale=1.0)
            for b in range(B_CORE):
                if b % 2 == 0:
                    nc.vector.tensor_copy(wp8[:, b, b:b + 1],
                                          w2_s[:, b:b + 1])
                else:
                    nc.scalar.activation(wp8[:, b, b:b + 1],
                                         w2_s[:, b:b + 1], AF.Identity,
                                         bias=z16[:, 0:1], scale=1.0)

            # ---------- pointer scores -> tanh -> exp -> norm ----------
            tp = bigp.tile([B_CORE, N2], F16, tag="tp")
            e2 = bigp.tile([B_CORE, N2], F32, tag="e2")
            s2s = smp.tile([B_CORE, NCH], F32, tag="s2s")
            s2p = smp.tile([B_CORE, 2], F32, tag="s2p")
            for c in range(NCH):
                xTc = xT[:, c, :, :]
                cs = slice(CW * c, CW * (c + 1))
                pc = psc.tile([H, CW], F32, tag="scps", name="pc")
                for b in range(B_CORE):
                    nc.tensor.matmul(pc[:], wp8[:, b, :], xTc[:, b, :],
                                     start=(b == 0), stop=False)
                nc.tensor.matmul(pc[:], mnegP9, oh9[:, cs],
                                 start=False, stop=True)
                nc.scalar.activation(tp[:, cs], pc[0:B_CORE, :], AF.Tanh)
                nc.scalar.activation(
                    e2[:, cs], tp[:, cs], AF.Exp,
                    bias=0.0, scale=10.0, accum_out=s2s[:, c:c + 1])
                if c == 1:
                    nc.vector.tensor_add(s2p[:, 0:1], s2s[:, 0:1],
                                         s2s[:, 1:2])
                if c == 3:
                    nc.vector.tensor_add(s2p[:, 1:2], s2s[:, 2:3],
                                         s2s[:, 3:4])

            s2tot = smp.tile([B_CORE, 1], F32, tag="s2tot")
            nc.vector.tensor_add(s2tot[:], s2p[:, 0:1], s2p[:, 1:2])
            rcp2 = smp.tile([B_CORE, 1], F32, tag="rcp2")
            nc.vector.reciprocal(rcp2[:], s2tot[:])
            NHALF = 1024
            nc.vector.tensor_scalar_mul(e2[:, 0:NHALF], e2[:, 0:NHALF],
                                        rcp2[:, 0:1])
            nc.sync.dma_start(probs[:, 0:NHALF], e2[:, 0:NHALF])
            nc.scalar.activation(e2[:, NHALF:N], e2[:, NHALF:N], AF.Identity,
                                 bias=0.0, scale=rcp2[:, 0:1])
            nc.sync.dma_start(probs[:, NHALF:N], e2[:, NHALF:N])

    nc.compile()
    return nc


def _prep_inputs(patch_embeddings, fixed_content_cls, Wq_graph, Wq_first,
                 Wq_last, Wk, Wv, logit_Wk, Wc, bc, last_patch):
    qs = 1.0 / math.sqrt(HD)
    ls = 1.0 / math.sqrt(H)
    f16 = lambda a: np.ascontiguousarray(a, dtype=np.float16)
    f32 = lambda a: np.ascontiguousarray(a, dtype=np.float32)

    hm = np.zeros((H, NH), np.float32)
    for h in range(NH):
        hm[HD * h:HD * (h + 1), h] = qs

    pe = np.asarray(patch_embeddings, dtype=np.float32)
    cls = np.asarray(fixed_content_cls, dtype=np.float32)
    lp = np.asarray(last_patch).astype(np.int64)
    lWk = np.asarray(logit_Wk, dtype=np.float64)
    Wc64 = np.asarray(Wc, dtype=np.float64)

    wpack_base = np.zeros((H, WPACK_W), np.float16)
    wpack_base[:, 0:128] = f16(np.asarray(Wq_graph).T)
    wpack_base[:, 128:256] = f16(np.asarray(Wq_first) + np.asarray(Wq_last)).T
    wpack_base[:, 256:384] = f16(Wk)
    wpack_base[:, 384:512] = f16(np.asarray(Wv).T)
    # w2 = wlc^T v + blc : lhsT = (ls * lWk^T Wc)^T = ls * Wc^T lWk
    wpack_base[:, 512:640] = f16(Wc64.T @ lWk * ls)
    wpack_base[:, 640:768] = np.eye(H, dtype=np.float16)
    wpack_base[:, 768:776] = f16(hm)

    blc = f32((lWk.T @ np.asarray(bc, dtype=np.float64) * ls)[:, None])

    in_maps = []
    for c in range(N_CORES):
        bs = slice(B_CORE * c, B_CORE * (c + 1))
        pec = pe[bs]                        # (8, 2000, 128)
        lp_c = lp[bs]
        wpack = wpack_base.copy()
        wpack[:, 776:784] = f16(pec[np.arange(B_CORE), lp_c].T)   # leT
        wpack[:, 784:792] = f16(cls[bs, 0, :].T)                   # clsT

        pad = np.zeros((B_CORE, N2, H), np.float16)
        pad[:, :N, :] = pec
        # (b, chunk, n, c) -> (c, chunk, b, n)
        xtp = np.ascontiguousarray(
            pad.reshape(B_CORE, NCH, CW, H).transpose(3, 1, 0, 2))

        small9 = np.zeros((B_CORE + 1, SM_W), np.float16)
        small9[np.arange(B_CORE), lp_c] = 1.0                      # one-hot
        small9[B_CORE, N:N2] = 1.0                                 # pad cols
        for b in range(B_CORE):
            small9[b, N2 + NH * b:N2 + NH * (b + 1)] = MNEG        # mnegA9
            small9[b, N2 + H + b] = MNEG                           # mnegP9
        small9[B_CORE, N2:N2 + 2 * H] = MNEG                       # pad row

        m = {"xtp": xtp, "wpack": wpack, "small9": small9, "blcd": blc}
        in_maps.append(m)
    return in_maps


def kernel(trace=False, **inputs):
    if "nc" not in _CACHE:
        _CACHE["nc"] = build()
    nc = _CACHE["nc"]
    in_maps = _prep_inputs(**inputs)
    res = run_bass_kernel_spmd(nc, in_maps, list(range(N_CORES)), trace=trace)
    out = np.concatenate(
        [res.results[c]["probs"].astype(np.float32) for c in range(N_CORES)],
        axis=0)
    if trace:
        return out, res
    return out
